# revision 7
# baseline (speedup 1.0000x reference)
"""Bahdanau additive attention on 8 Trainium2 NeuronCores (Bass/Tile).

Reference computation (per batch b):
    wq   = query @ wa_w.T + wa_b                      # [1, H]
    uk   = keys  @ ua_w.T + ua_b                      # [L, H]
    s    = tanh(wq + uk) @ va_w.T + va_b              # [L]
    s    = where(mask, -inf, s)
    w    = softmax(s)                                 # [L]
    ctx  = w @ keys                                   # [1, H]

Sharding: data-parallel over batch B=32 -> 4 batches per core; small
weights replicated.  The heavy matmul runs on the PE in bf16 (same
78.6 TF/s PE rate as fp32r, but half the DMA/SBUF footprint and 4x
faster weight loads via FWL; accuracy ~1e-3 rel, well inside 2e-2).

Device-side structure (per core, BC=4 batches):
  - keys arrive pre-transposed (host) as keysTr [128, HC, L] per batch;
    the big matmul computes uk^T [k, l] so the per-batch
    wq[k]+wa_b[k]+ua_b[k] (tiny, host-precomputed) is a per-partition
    ACT bias fused into the tanh.
  - scores = va . tanh(.) is a PE matmul with va as a [128,1] stationary.
  - softmax needs NO max subtraction: |scores| <= ||va||_1 ~ 26 << 88,
    so exp never overflows fp32.  exp + per-tile sum fuse into one ACT
    op (accum_out); softmax shift invariance drops va_b.
  - the weighted key sum ctx^T = sum_l e_l * keysT[:, l] runs on the DVE
    (scalar_tensor_tensor multiply with accum_out) against the SAME
    keysTr tiles pass 1 just consumed -> keys are read from HBM once.
    exp weights are partition-broadcast via a tiny PE ones-matmul.
  - tile 0 consumes weights hc-major (4 open PSUM accumulations) so the
    first matmul needs only uawT[hc0]+kT0[hc0] (~384KB) instead of the
    whole weight set; the PE starts ~1.5us in and HAM warms early.
  - per-batch outputs are the unnormalized ctx^T [128, HC] and the
    per-tile exp sums; the host divides by their total and transposes
    during the gather/unshard step (a 32 KB epilogue).
"""

import os
import numpy as np
from contextlib import ExitStack

import ml_dtypes

import concourse.bass as bass  # noqa: F401
import concourse.bacc as bacc
import concourse.tile as tile
from concourse import mybir
from concourse.bass_utils import run_bass_kernel_spmd

B, L, H = 32, 2048, 1024
NCORES = 8
BC = B // NCORES          # batches per core
HC = H // 128             # 128-chunks of the hidden dim
NSLOT = 8                 # slot-dim padding (last batch uses 5 slots)

# l-tile widths per batch; last batch ends with a small tile so the
# serial flush chain after the final matmul is short.
WIDTHS = [[512, 512, 512, 512]] * (BC - 1) + [[512, 512, 512, 384, 128]]

F32 = mybir.dt.float32
BF = mybir.dt.bfloat16
AF = mybir.ActivationFunctionType
AX = mybir.AxisListType
OP = mybir.AluOpType

_nc = None
LAST_RESULT = None


def _body(nc, tc, ctx, d):
    consts = ctx.enter_context(tc.tile_pool(name="consts", bufs=1))
    kpool = ctx.enter_context(tc.tile_pool(name="kT", bufs=8))
    tpool = ctx.enter_context(tc.tile_pool(name="tk", bufs=12))
    small = ctx.enter_context(tc.tile_pool(name="small", bufs=2))
    p_uk = ctx.enter_context(tc.tile_pool(name="p_uk", bufs=5, space="PSUM"))
    p_sc = ctx.enter_context(tc.tile_pool(name="p_sc", bufs=2, space="PSUM"))
    p_wb = ctx.enter_context(tc.tile_pool(name="p_wb", bufs=1, space="PSUM"))

    # ---- weights on the (otherwise idle) GPSIMD HWDGE queue so neither
    # the keysTr stream (sync queue) nor the ACT engine is delayed.
    # One tile per hc chunk => the first matmul depends only on chunk 0,
    # not the whole 2MB weight set. ----
    uaw = []
    with tc.high_priority():
        for hc in range(HC):
            w = consts.tile([128, H], BF, name=f"uaw{hc}")
            nc.gpsimd.dma_start(w[:], d["uawT"][:, hc * H : (hc + 1) * H])
            uaw.append(w)
        biasT = consts.tile([128, HC * BC], F32)
        nc.gpsimd.dma_start(biasT[:], d["biasT"])
        vaT = consts.tile([128, HC], BF)
        nc.gpsimd.dma_start(vaT[:], d["vaT"])
        ones_r = consts.tile([1, 128], BF)
        nc.gpsimd.dma_start(ones_r[:], d["ones"])

    # Per-batch state, created lazily inside the flat tile loop.
    bstate = {}

    def batch_state(b):
        if b not in bstate:
            mb = small.tile([1, L], F32, tag="mb")
            nc.gpsimd.dma_start(mb[:], d["maskb"][b : b + 1, :])
            s_all = small.tile([1, NSLOT], F32, tag="s_all", name=f"s_all_{b}")
            pp_all = small.tile([128, HC, NSLOT], F32, tag="pp_all",
                                name=f"pp_all_{b}")
            bstate[b] = {"mb": mb, "s_all": s_all, "pp_all": pp_all}
        return bstate[b]

    def emit_scores(rec, kc):
        """One deferred score matmul for tile rec at chunk kc (its tanh is
        a full tile old, so this never stalls the PE)."""
        nc.tensor.matmul(
            rec["ps"][:], vaT[:, kc : kc + 1], rec["tks"][kc][:],
            start=(kc == 0), stop=(kc == HC - 1),
        )

    def emit_softmax(rec):
        """Mask add + exp(+sum) for tile rec; DVE/ACT only.  No max
        subtraction: scores are bounded by ||va||_1 << fp32 exp range."""
        b, l0, lw = rec["b"], rec["l0"], rec["lw"]
        st = bstate[b]
        sm = small.tile([1, lw], F32, tag="sm")
        nc.vector.tensor_add(sm[:], rec["ps"][:],
                             st["mb"][0:1, l0 : l0 + lw])
        e = small.tile([1, lw], BF, tag="e", bufs=3, name=f"e_{b}_{rec['lt']}")
        nc.scalar.activation(e[:], sm[:], AF.Exp, bias=0.0, scale=1.0,
                             accum_out=st["s_all"][0:1, rec["slot"] : rec["slot"] + 1])
        rec["e"] = e

    def emit_wbcast(rec):
        """Partition-broadcast of the exp weights: tiny PE ones-matmul,
        then an ACT copy out of PSUM into a bf16 SBUF tile."""
        lw = rec["lw"]
        wb = p_wb.tile([128, lw], F32, tag="wb")
        nc.tensor.matmul(wb[:], ones_r[:], rec["e"][:], start=True, stop=True)
        wbs = small.tile([128, lw], BF, tag="wbs", bufs=2,
                         name=f"wbs_{rec['b']}_{rec['lt']}")
        nc.scalar.activation(wbs[:], wb[:], AF.Copy)
        rec["wbs"] = wbs

    def emit_wsum(rec):
        """DVE weighted key sum against the resident keysTr tile."""
        b = rec["b"]
        st = bstate[b]
        for hc in range(HC):
            dump = small.tile([128, rec["lw"]], BF, tag="dump")
            nc.vector.scalar_tensor_tensor(
                dump[:],
                rec["kTs"][hc],
                1.0,
                rec["wbs"][:],
                op0=OP.mult,
                op1=OP.mult,
                accum_out=st["pp_all"][:, hc, rec["slot"] : rec["slot"] + 1],
            )
        if rec["last"]:
            ns = rec["slot"] + 1
            acc = small.tile([128, HC], F32, tag="acc")
            nc.vector.tensor_reduce(acc[:], st["pp_all"][:, :, 0:ns],
                                    axis=AX.X, op=OP.add)
            nc.gpsimd.dma_start(d["accout"][b, :, :], acc[:])
            nc.gpsimd.dma_start(d["sout"][b : b + 1, 0:ns], st["s_all"][0:1, 0:ns])

    # tile plan: flat list of (b, l0, lw)
    plan = []
    for b in range(BC):
        l0 = 0
        for i, w in enumerate(WIDTHS[b]):
            plan.append({"b": b, "lt": i, "slot": i, "l0": l0, "lw": w,
                         "last": i == len(WIDTHS[b]) - 1})
            l0 += w

    tiles = []
    for t, rec in enumerate(plan):
        b, l0, lw = rec["b"], rec["l0"], rec["lw"]
        batch_state(b)
        if t == 0:
            # separate per-hc chunk tiles so the hc-major warmup's first
            # matmul waits only on uaw[0]+kTs[0] (~384KB), not 3MB
            kTs = []
            with tc.high_priority():
                for hc in range(HC):
                    kc0 = kpool.tile([128, lw], BF, name=f"kT0c{hc}")
                    nc.sync.dma_start(kc0[:], d["keysTr"][b, :, hc, l0 : l0 + lw])
                    kTs.append(kc0[:])
        else:
            kT = kpool.tile([128, HC, lw], BF, tag="kT")
            with tc.high_priority():
                nc.sync.dma_start(kT[:, :, :], d["keysTr"][b, :, :, l0 : l0 + lw])
            kTs = [kT[:, hc, :] for hc in range(HC)]
        ps = p_sc.tile([1, lw], F32, tag="ps")
        rec.update({"kTs": kTs, "tks": [], "ps": ps})

        def mm(pu, kc, hc):
            nc.tensor.matmul(
                pu[:],
                uaw[hc][:, kc * 128 : (kc + 1) * 128],
                kTs[hc],
                start=(hc == 0),
                stop=(hc == HC - 1),
            )

        def tanh(kc, pu):
            tk = tpool.tile([128, lw], BF, tag="tk")
            nc.scalar.activation(
                tk[:], pu[:], AF.Tanh,
                bias=biasT[:, kc * BC + b : kc * BC + b + 1], scale=1.0,
            )
            rec["tks"].append(tk)

        if t == 0:
            # warm-up: hc-major over kc 0..3 (4 open PSUM accumulations),
            # so compute starts as soon as uawT[hc0]+kT0[hc0] land.
            pus = [p_uk.tile([128, lw], F32, tag="pu", name=f"pu_w{kc}")
                   for kc in range(4)]
            for hc in range(HC):
                for kc in range(4):
                    mm(pus[kc], kc, hc)
            # pass B: kc 4..7 kc-major (weights all resident by now),
            # pipelined with pass A's tanhs.
            for kc in range(4, HC):
                pu = p_uk.tile([128, lw], F32, tag="pu")
                for hc in range(HC):
                    mm(pu, kc, hc)
                tanh(kc - 4, pus[kc - 4])
                pus.append(pu)
            for kc in range(4, HC):
                tanh(kc, pus[kc])
        else:
            for kc in range(HC):
                pu = p_uk.tile([128, lw], F32, tag="pu")
                for hc in range(HC):
                    mm(pu, kc, hc)
                tanh(kc, pu)
                prev = tiles[t - 1]
                if kc < 4:
                    emit_scores(prev, 2 * kc)
                    emit_scores(prev, 2 * kc + 1)
                elif kc == 4:
                    emit_softmax(prev)
                elif kc == 5:
                    emit_wbcast(prev)
                elif kc == HC - 1:
                    emit_wsum(prev)
                if t == len(plan) - 1 and kc >= 1:
                    # final tile: its own scores lag one kc slot so the
                    # flush chain is only tanh(7) -> sc(7) -> exp -> wsum
                    emit_scores(rec, kc - 1)
        tiles.append(rec)

    # flush the final tile
    last = tiles[-1]
    emit_scores(last, HC - 1)
    emit_softmax(last)
    emit_wbcast(last)
    emit_wsum(last)


def build():
    nc = bacc.Bacc("TRN2", target_bir_lowering=False, debug=False,
                   num_devices=NCORES)
    d = {
        "keysTr": nc.dram_tensor("keysTr", [BC, 128, HC, L], BF, kind="ExternalInput").ap(),
        "uawT": nc.dram_tensor("uawT", [128, HC * H], BF, kind="ExternalInput").ap(),
        "vaT": nc.dram_tensor("vaT", [128, HC], BF, kind="ExternalInput").ap(),
        "biasT": nc.dram_tensor("biasT", [128, HC * BC], F32, kind="ExternalInput").ap(),
        "ones": nc.dram_tensor("ones", [1, 128], BF, kind="ExternalInput").ap(),
        "maskb": nc.dram_tensor("maskb", [BC, L], F32, kind="ExternalInput").ap(),
        "accout": nc.dram_tensor("accout", [BC, 128, HC], F32, kind="ExternalOutput").ap(),
        "sout": nc.dram_tensor("sout", [BC, NSLOT], F32, kind="ExternalOutput").ap(),
    }
    with tile.TileContext(nc) as tc, ExitStack() as ctx:
        _body(nc, tc, ctx, d)
    nc.compile()
    return nc


def _maybe_install_profile_hook():
    """BASS_TRACE=1 profiling under axon needs antenv.axon_hooks, which this
    image lacks; shim it with an in-memory module wired to libaxon_pjrt."""
    import sys, types
    if "antenv.axon_hooks" in sys.modules:
        return
    mod = types.ModuleType("antenv.axon_hooks")
    holder = [None]
    mod.set_axon_ntff_profile_hook = lambda h: holder.__setitem__(0, h)
    mod.get_axon_ntff_profile_hook = lambda: holder[0]
    sys.modules["antenv.axon_hooks"] = mod
    try:
        from trn_agent_boot.trn_boot import _ntff_profile_via_ctypes
        mod.set_axon_ntff_profile_hook(
            _ntff_profile_via_ctypes("/opt/axon/libaxon_pjrt.so"))
    except Exception:
        pass


def make_in_maps(query, keys, mask, wa_w, wa_b, ua_w, ua_b, va_w, va_b):
    bf16 = ml_dtypes.bfloat16
    query = np.asarray(query, dtype=np.float32)
    keys = np.asarray(keys, dtype=np.float32)
    mask = np.asarray(mask)
    wa_w = np.asarray(wa_w, dtype=np.float32)
    wa_b = np.asarray(wa_b, dtype=np.float32)
    ua_b = np.asarray(ua_b, dtype=np.float32)
    ua_w = np.asarray(ua_w, dtype=np.float32)
    va_w = np.asarray(va_w, dtype=np.float32)

    # lhsT chunk layout: arr[p, hc*H + k] = W[k, hc*128 + p]
    uawT = np.ascontiguousarray(
        ua_w.T.reshape(HC, 128, H).transpose(1, 0, 2).reshape(128, HC * H)
    ).astype(bf16)
    vaT = np.ascontiguousarray(va_w[0].reshape(HC, 128).T).astype(bf16)
    maskb = np.where(mask, np.float32(-1e30), np.float32(0.0)).astype(np.float32)
    # keysTr[b, p, hc, l] = keys[b, l, hc*128+p]
    keysTr = np.ascontiguousarray(
        keys.transpose(0, 2, 1).reshape(B, HC, 128, L).transpose(0, 2, 1, 3)
    ).astype(bf16)
    # wq + wa_b + ua_b on host (0.05% of the FLOPs)
    wq = query[:, 0, :] @ wa_w.T + wa_b + ua_b  # [B, H]

    in_maps = []
    for c in range(NCORES):
        bs = slice(c * BC, (c + 1) * BC)
        biasT = np.ascontiguousarray(
            wq[bs].T.reshape(HC, 128, BC).transpose(1, 0, 2).reshape(128, HC * BC))
        in_maps.append({
            "keysTr": keysTr[bs],
            "uawT": uawT,
            "vaT": vaT,
            "biasT": biasT,
            "ones": np.ones((1, 128), dtype=bf16),
            "maskb": np.ascontiguousarray(maskb[bs]),
        })
    return in_maps


def kernel(query, keys, mask, wa_w, wa_b, ua_w, ua_b, va_w, va_b):
    global _nc, LAST_RESULT
    if os.environ.get("BASS_TRACE"):
        _maybe_install_profile_hook()
    if _nc is None:
        _nc = build()
    in_maps = make_in_maps(query, keys, mask, wa_w, wa_b, ua_w, ua_b, va_w, va_b)
    res = run_bass_kernel_spmd(_nc, in_maps, list(range(NCORES)))
    LAST_RESULT = res
    outs = []
    for c in range(NCORES):
        acc = res.results[c]["accout"]          # [BC, 128, HC] = ctx^T unnormalized
        sout = res.results[c]["sout"]  # [BC, NSLOT]; batch i uses len(WIDTHS[i]) slots
        ssum = np.array([sout[i, : len(WIDTHS[i])].sum() for i in range(BC)])
        # ctx[b, hc*128+p] = acc[b, p, hc] / ssum[b]
        ctx = acc.transpose(0, 2, 1).reshape(BC, H) / ssum[:, None]
        outs.append(ctx)
    out = np.concatenate(outs, axis=0)
    return np.ascontiguousarray(out[:, None, :].astype(np.float32))


# revision 8
# speedup vs baseline: 1.0768x; 1.0768x over previous
"""Bahdanau additive attention on 8 Trainium2 NeuronCores (Bass/Tile).

Reference computation (per batch b):
    wq   = query @ wa_w.T + wa_b                      # [1, H]
    uk   = keys  @ ua_w.T + ua_b                      # [L, H]
    s    = tanh(wq + uk) @ va_w.T + va_b              # [L]
    s    = where(mask, -inf, s)
    w    = softmax(s)                                 # [L]
    ctx  = w @ keys                                   # [1, H]

Sharding: data-parallel over batch B=32 -> 4 batches per core; small
weights replicated.  The heavy matmul runs on the PE in bf16 (same
78.6 TF/s PE rate as fp32r, but half the DMA/SBUF footprint and 4x
faster weight loads via FWL; accuracy ~1e-3 rel, well inside 2e-2).

Device-side structure (per core, BC=4 batches):
  - keys arrive pre-transposed (host) as keysTr [128, HC, L] per batch;
    the big matmul computes uk^T [k, l] so the per-batch
    wq[k]+wa_b[k]+ua_b[k] (tiny, host-precomputed) is a per-partition
    ACT bias fused into the tanh.
  - scores = va . tanh(.) is a PE matmul with va as a [128,1] stationary.
  - softmax needs NO max subtraction: |scores| <= ||va||_1 ~ 26 << 88,
    so exp never overflows fp32.  exp + per-tile sum fuse into one ACT
    op (accum_out); softmax shift invariance drops va_b.
  - the weighted key sum ctx^T = sum_l e_l * keysT[:, l] runs on the DVE
    (scalar_tensor_tensor multiply with accum_out) against the SAME
    keysTr tiles pass 1 just consumed -> keys are read from HBM once.
    exp weights are partition-broadcast via a tiny PE ones-matmul.
  - tile 0 consumes weights hc-major (4 open PSUM accumulations) so the
    first matmul needs only uawT[hc0]+kT0[hc0] (~384KB) instead of the
    whole weight set; the PE starts ~1.5us in and HAM warms early.
  - per-batch outputs are the unnormalized ctx^T [128, HC] and the
    per-tile exp sums; the host divides by their total and transposes
    during the gather/unshard step (a 32 KB epilogue).
"""

import os
import numpy as np
from contextlib import ExitStack

import ml_dtypes

import concourse.bass as bass  # noqa: F401
import concourse.bacc as bacc
import concourse.tile as tile
from concourse import mybir
from concourse.bass_utils import run_bass_kernel_spmd

B, L, H = 32, 2048, 1024
NCORES = 8
BC = B // NCORES          # batches per core
HC = H // 128             # 128-chunks of the hidden dim
NSLOT = 8                 # slot-dim padding (last batch uses 5 slots)

# l-tile widths per batch; last batch ends with a small tile so the
# serial flush chain after the final matmul is short.
WIDTHS = [[512, 512, 512, 512]] * (BC - 1) + [[512, 512, 512, 384, 128]]

F32 = mybir.dt.float32
BF = mybir.dt.bfloat16
AF = mybir.ActivationFunctionType
AX = mybir.AxisListType
OP = mybir.AluOpType

_nc = None
LAST_RESULT = None


def _body(nc, tc, ctx, d):
    consts = ctx.enter_context(tc.tile_pool(name="consts", bufs=1))
    kpool = ctx.enter_context(tc.tile_pool(name="kT", bufs=8))
    tpool = ctx.enter_context(tc.tile_pool(name="tk", bufs=12))
    small = ctx.enter_context(tc.tile_pool(name="small", bufs=2))
    p_uk = ctx.enter_context(tc.tile_pool(name="p_uk", bufs=5, space="PSUM"))
    p_sc = ctx.enter_context(tc.tile_pool(name="p_sc", bufs=2, space="PSUM"))
    p_wb = ctx.enter_context(tc.tile_pool(name="p_wb", bufs=1, space="PSUM"))

    # ---- weights on the (otherwise idle) GPSIMD HWDGE queue so neither
    # the keysTr stream (sync queue) nor the ACT engine is delayed.
    # One tile per hc chunk => the first matmul depends only on chunk 0,
    # not the whole 2MB weight set. ----
    uaw = []
    with tc.high_priority():
        for hc in range(HC):
            w = consts.tile([128, H], BF, name=f"uaw{hc}")
            nc.gpsimd.dma_start(w[:], d["uawT"][:, hc * H : (hc + 1) * H])
            uaw.append(w)
        biasT = consts.tile([128, HC * BC], F32)
        nc.gpsimd.dma_start(biasT[:], d["biasT"])
        vaT = consts.tile([128, HC], BF)
        nc.gpsimd.dma_start(vaT[:], d["vaT"])
        ones_r = consts.tile([1, 128], BF)
        nc.gpsimd.dma_start(ones_r[:], d["ones"])

    # Per-batch state, created lazily inside the flat tile loop.
    bstate = {}

    def batch_state(b):
        if b not in bstate:
            mb = small.tile([1, L], F32, tag="mb")
            nc.gpsimd.dma_start(mb[:], d["maskb"][b : b + 1, :])
            s_all = small.tile([1, NSLOT], F32, tag="s_all", name=f"s_all_{b}")
            pp_all = small.tile([128, HC, NSLOT], F32, tag="pp_all",
                                name=f"pp_all_{b}")
            bstate[b] = {"mb": mb, "s_all": s_all, "pp_all": pp_all}
        return bstate[b]

    def emit_scores(rec, kc):
        """One deferred score matmul for tile rec at chunk kc (its tanh is
        a full tile old, so this never stalls the PE)."""
        nc.tensor.matmul(
            rec["ps"][:], vaT[:, kc : kc + 1], rec["tks"][kc][:],
            start=(kc == 0), stop=(kc == HC - 1),
        )

    def emit_softmax(rec):
        """Mask add + exp(+sum) for tile rec; DVE/ACT only.  No max
        subtraction: scores are bounded by ||va||_1 << fp32 exp range."""
        b, l0, lw = rec["b"], rec["l0"], rec["lw"]
        st = bstate[b]
        sm = small.tile([1, lw], F32, tag="sm")
        nc.vector.tensor_add(sm[:], rec["ps"][:],
                             st["mb"][0:1, l0 : l0 + lw])
        e = small.tile([1, lw], BF, tag="e", bufs=3, name=f"e_{b}_{rec['lt']}")
        nc.scalar.activation(e[:], sm[:], AF.Exp, bias=0.0, scale=1.0,
                             accum_out=st["s_all"][0:1, rec["slot"] : rec["slot"] + 1])
        rec["e"] = e

    def emit_wbcast(rec):
        """Partition-broadcast of the exp weights: tiny PE ones-matmul,
        then an ACT copy out of PSUM into a bf16 SBUF tile."""
        lw = rec["lw"]
        wb = p_wb.tile([128, lw], F32, tag="wb")
        nc.tensor.matmul(wb[:], ones_r[:], rec["e"][:], start=True, stop=True)
        wbs = small.tile([128, lw], BF, tag="wbs", bufs=2,
                         name=f"wbs_{rec['b']}_{rec['lt']}")
        nc.scalar.activation(wbs[:], wb[:], AF.Copy)
        rec["wbs"] = wbs

    def emit_wsum(rec):
        """DVE weighted key sum against the resident keysTr tile."""
        b = rec["b"]
        st = bstate[b]
        for hc in range(HC):
            dump = small.tile([128, rec["lw"]], BF, tag="dump")
            nc.vector.scalar_tensor_tensor(
                dump[:],
                rec["kTs"][hc],
                1.0,
                rec["wbs"][:],
                op0=OP.mult,
                op1=OP.mult,
                accum_out=st["pp_all"][:, hc, rec["slot"] : rec["slot"] + 1],
            )
        if rec["last"]:
            ns = rec["slot"] + 1
            acc = small.tile([128, HC], F32, tag="acc")
            nc.vector.tensor_reduce(acc[:], st["pp_all"][:, :, 0:ns],
                                    axis=AX.X, op=OP.add)
            nc.gpsimd.dma_start(d["accout"][b, :, :], acc[:])
            nc.gpsimd.dma_start(d["sout"][b : b + 1, 0:ns], st["s_all"][0:1, 0:ns])

    # tile plan: flat list of (b, l0, lw)
    plan = []
    for b in range(BC):
        l0 = 0
        for i, w in enumerate(WIDTHS[b]):
            plan.append({"b": b, "lt": i, "slot": i, "l0": l0, "lw": w,
                         "last": i == len(WIDTHS[b]) - 1})
            l0 += w

    tiles = []
    for t, rec in enumerate(plan):
        b, l0, lw = rec["b"], rec["l0"], rec["lw"]
        batch_state(b)
        if t == 0:
            # separate per-hc chunk tiles so the hc-major warmup's first
            # matmul waits only on uaw[0]+kTs[0] (~384KB), not 3MB
            kTs = []
            with tc.high_priority():
                for hc in range(HC):
                    kc0 = kpool.tile([128, lw], BF, name=f"kT0c{hc}")
                    nc.sync.dma_start(kc0[:], d["keysTr"][b, :, hc, l0 : l0 + lw])
                    kTs.append(kc0[:])
        else:
            kT = kpool.tile([128, HC, lw], BF, tag="kT")
            if t == 1:
                # kT1 must land before tile-0 compute drains (~15us); the
                # scheduler's own model places it too late.  Later tiles
                # ride the steady-state pipeline - prioritizing them only
                # floods HBM and starves the warmup path.
                with tc.high_priority():
                    nc.sync.dma_start(kT[:, :, :], d["keysTr"][b, :, :, l0 : l0 + lw])
            else:
                nc.sync.dma_start(kT[:, :, :], d["keysTr"][b, :, :, l0 : l0 + lw])
            kTs = [kT[:, hc, :] for hc in range(HC)]
        ps = p_sc.tile([1, lw], F32, tag="ps")
        rec.update({"kTs": kTs, "tks": [], "ps": ps})

        def mm(pu, kc, hc):
            nc.tensor.matmul(
                pu[:],
                uaw[hc][:, kc * 128 : (kc + 1) * 128],
                kTs[hc],
                start=(hc == 0),
                stop=(hc == HC - 1),
            )

        def tanh(kc, pu):
            tk = tpool.tile([128, lw], BF, tag="tk")
            nc.scalar.activation(
                tk[:], pu[:], AF.Tanh,
                bias=biasT[:, kc * BC + b : kc * BC + b + 1], scale=1.0,
            )
            rec["tks"].append(tk)

        if t == 0:
            # warm-up: hc-major over kc 0..3 (4 open PSUM accumulations),
            # so compute starts as soon as uawT[hc0]+kT0[hc0] land.
            pus = [p_uk.tile([128, lw], F32, tag="pu", name=f"pu_w{kc}")
                   for kc in range(4)]
            for hc in range(HC):
                for kc in range(4):
                    mm(pus[kc], kc, hc)
            # pass B: kc 4..7 kc-major (weights all resident by now),
            # pipelined with pass A's tanhs.
            for kc in range(4, HC):
                pu = p_uk.tile([128, lw], F32, tag="pu")
                for hc in range(HC):
                    mm(pu, kc, hc)
                tanh(kc - 4, pus[kc - 4])
                pus.append(pu)
            for kc in range(4, HC):
                tanh(kc, pus[kc])
        else:
            for kc in range(HC):
                pu = p_uk.tile([128, lw], F32, tag="pu")
                for hc in range(HC):
                    mm(pu, kc, hc)
                tanh(kc, pu)
                prev = tiles[t - 1]
                if kc < 4:
                    emit_scores(prev, 2 * kc)
                    emit_scores(prev, 2 * kc + 1)
                elif kc == 4:
                    emit_softmax(prev)
                elif kc == 5:
                    emit_wbcast(prev)
                elif kc == HC - 1:
                    emit_wsum(prev)
                if t == len(plan) - 1 and kc >= 1:
                    # final tile: its own scores lag one kc slot so the
                    # flush chain is only tanh(7) -> sc(7) -> exp -> wsum
                    emit_scores(rec, kc - 1)
        tiles.append(rec)

    # flush the final tile
    last = tiles[-1]
    emit_scores(last, HC - 1)
    emit_softmax(last)
    emit_wbcast(last)
    emit_wsum(last)


def build():
    nc = bacc.Bacc("TRN2", target_bir_lowering=False, debug=False,
                   num_devices=NCORES)
    d = {
        "keysTr": nc.dram_tensor("keysTr", [BC, 128, HC, L], BF, kind="ExternalInput").ap(),
        "uawT": nc.dram_tensor("uawT", [128, HC * H], BF, kind="ExternalInput").ap(),
        "vaT": nc.dram_tensor("vaT", [128, HC], BF, kind="ExternalInput").ap(),
        "biasT": nc.dram_tensor("biasT", [128, HC * BC], F32, kind="ExternalInput").ap(),
        "ones": nc.dram_tensor("ones", [1, 128], BF, kind="ExternalInput").ap(),
        "maskb": nc.dram_tensor("maskb", [BC, L], F32, kind="ExternalInput").ap(),
        "accout": nc.dram_tensor("accout", [BC, 128, HC], F32, kind="ExternalOutput").ap(),
        "sout": nc.dram_tensor("sout", [BC, NSLOT], F32, kind="ExternalOutput").ap(),
    }
    with tile.TileContext(nc) as tc, ExitStack() as ctx:
        _body(nc, tc, ctx, d)
    nc.compile()
    return nc


def _maybe_install_profile_hook():
    """BASS_TRACE=1 profiling under axon needs antenv.axon_hooks, which this
    image lacks; shim it with an in-memory module wired to libaxon_pjrt."""
    import sys, types
    if "antenv.axon_hooks" in sys.modules:
        return
    mod = types.ModuleType("antenv.axon_hooks")
    holder = [None]
    mod.set_axon_ntff_profile_hook = lambda h: holder.__setitem__(0, h)
    mod.get_axon_ntff_profile_hook = lambda: holder[0]
    sys.modules["antenv.axon_hooks"] = mod
    try:
        from trn_agent_boot.trn_boot import _ntff_profile_via_ctypes
        mod.set_axon_ntff_profile_hook(
            _ntff_profile_via_ctypes("/opt/axon/libaxon_pjrt.so"))
    except Exception:
        pass


def make_in_maps(query, keys, mask, wa_w, wa_b, ua_w, ua_b, va_w, va_b):
    bf16 = ml_dtypes.bfloat16
    query = np.asarray(query, dtype=np.float32)
    keys = np.asarray(keys, dtype=np.float32)
    mask = np.asarray(mask)
    wa_w = np.asarray(wa_w, dtype=np.float32)
    wa_b = np.asarray(wa_b, dtype=np.float32)
    ua_b = np.asarray(ua_b, dtype=np.float32)
    ua_w = np.asarray(ua_w, dtype=np.float32)
    va_w = np.asarray(va_w, dtype=np.float32)

    # lhsT chunk layout: arr[p, hc*H + k] = W[k, hc*128 + p]
    uawT = np.ascontiguousarray(
        ua_w.T.reshape(HC, 128, H).transpose(1, 0, 2).reshape(128, HC * H)
    ).astype(bf16)
    vaT = np.ascontiguousarray(va_w[0].reshape(HC, 128).T).astype(bf16)
    maskb = np.where(mask, np.float32(-1e30), np.float32(0.0)).astype(np.float32)
    # keysTr[b, p, hc, l] = keys[b, l, hc*128+p]
    keysTr = np.ascontiguousarray(
        keys.transpose(0, 2, 1).reshape(B, HC, 128, L).transpose(0, 2, 1, 3)
    ).astype(bf16)
    # wq + wa_b + ua_b on host (0.05% of the FLOPs)
    wq = query[:, 0, :] @ wa_w.T + wa_b + ua_b  # [B, H]

    in_maps = []
    for c in range(NCORES):
        bs = slice(c * BC, (c + 1) * BC)
        biasT = np.ascontiguousarray(
            wq[bs].T.reshape(HC, 128, BC).transpose(1, 0, 2).reshape(128, HC * BC))
        in_maps.append({
            "keysTr": keysTr[bs],
            "uawT": uawT,
            "vaT": vaT,
            "biasT": biasT,
            "ones": np.ones((1, 128), dtype=bf16),
            "maskb": np.ascontiguousarray(maskb[bs]),
        })
    return in_maps


def kernel(query, keys, mask, wa_w, wa_b, ua_w, ua_b, va_w, va_b):
    global _nc, LAST_RESULT
    if os.environ.get("BASS_TRACE"):
        _maybe_install_profile_hook()
    if _nc is None:
        _nc = build()
    in_maps = make_in_maps(query, keys, mask, wa_w, wa_b, ua_w, ua_b, va_w, va_b)
    res = run_bass_kernel_spmd(_nc, in_maps, list(range(NCORES)))
    LAST_RESULT = res
    outs = []
    for c in range(NCORES):
        acc = res.results[c]["accout"]          # [BC, 128, HC] = ctx^T unnormalized
        sout = res.results[c]["sout"]  # [BC, NSLOT]; batch i uses len(WIDTHS[i]) slots
        ssum = np.array([sout[i, : len(WIDTHS[i])].sum() for i in range(BC)])
        # ctx[b, hc*128+p] = acc[b, p, hc] / ssum[b]
        ctx = acc.transpose(0, 2, 1).reshape(BC, H) / ssum[:, None]
        outs.append(ctx)
    out = np.concatenate(outs, axis=0)
    return np.ascontiguousarray(out[:, None, :].astype(np.float32))


# revision 14
# speedup vs baseline: 1.1011x; 1.0226x over previous
"""Bahdanau additive attention on 8 Trainium2 NeuronCores (Bass/Tile).

Reference computation (per batch b):
    wq   = query @ wa_w.T + wa_b                      # [1, H]
    uk   = keys  @ ua_w.T + ua_b                      # [L, H]
    s    = tanh(wq + uk) @ va_w.T + va_b              # [L]
    s    = where(mask, -inf, s)
    w    = softmax(s)                                 # [L]
    ctx  = w @ keys                                   # [1, H]

Sharding: data-parallel over batch B=32 -> 4 batches per core; small
weights replicated.  The heavy matmul runs on the PE in bf16 (same
78.6 TF/s PE rate as fp32r, but half the DMA/SBUF footprint and 4x
faster weight loads via FWL; accuracy ~1e-3 rel, well inside 2e-2).

Device-side structure (per core, BC=4 batches):
  - keys arrive pre-transposed (host) as keysTr [128, HC, L] per batch;
    the big matmul computes uk^T [k, l] so the per-batch
    wq[k]+wa_b[k]+ua_b[k] (tiny, host-precomputed) is a per-partition
    ACT bias fused into the tanh.
  - scores = va . tanh(.) is a PE matmul with va as a [128,1] stationary.
  - softmax needs NO max subtraction: |scores| <= ||va||_1 ~ 26 << 88,
    so exp never overflows fp32.  exp + per-tile sum fuse into one ACT
    op (accum_out); softmax shift invariance drops va_b.
  - the weighted key sum ctx^T = sum_l e_l * keysT[:, l] runs on the DVE
    (scalar_tensor_tensor multiply with accum_out) against the SAME
    keysTr tiles pass 1 just consumed -> keys are read from HBM once.
    exp weights are partition-broadcast via a tiny PE ones-matmul.
  - tile 0 consumes weights hc-major (4 open PSUM accumulations) so the
    first matmul needs only uawT[hc0]+kT0[hc0] (~384KB) instead of the
    whole weight set; the PE starts ~1.5us in and HAM warms early.
  - per-batch outputs are the unnormalized ctx^T [128, HC] and the
    per-tile exp sums; the host divides by their total and transposes
    during the gather/unshard step (a 32 KB epilogue).
"""

import os
import numpy as np
from contextlib import ExitStack

import ml_dtypes

import concourse.bass as bass  # noqa: F401
import concourse.bacc as bacc
import concourse.tile as tile
from concourse import mybir
from concourse.bass_utils import run_bass_kernel_spmd

B, L, H = 32, 2048, 1024
NCORES = 8
BC = B // NCORES          # batches per core
HC = H // 128             # 128-chunks of the hidden dim
NSLOT = 8                 # slot-dim padding (last batch uses 5 slots)

# l-tile widths per batch; last batch tapers so (a) the second-to-last
# tile's DVE weighted-sum hides under the last tile's matmul stream and
# (b) the serial flush chain after the final matmul is short.
WIDTHS = [[512, 512, 512, 512]] * (BC - 1) + [[512, 512, 384, 384, 256]]

F32 = mybir.dt.float32
BF = mybir.dt.bfloat16
AF = mybir.ActivationFunctionType
AX = mybir.AxisListType
OP = mybir.AluOpType

_nc = None
LAST_RESULT = None


def _body(nc, tc, ctx, d):
    consts = ctx.enter_context(tc.tile_pool(name="consts", bufs=1))
    kpool = ctx.enter_context(tc.tile_pool(name="kT", bufs=8))
    tpool = ctx.enter_context(tc.tile_pool(name="tk", bufs=12))
    small = ctx.enter_context(tc.tile_pool(name="small", bufs=2))
    p_uk = ctx.enter_context(tc.tile_pool(name="p_uk", bufs=5, space="PSUM"))
    p_sc = ctx.enter_context(tc.tile_pool(name="p_sc", bufs=2, space="PSUM"))
    p_wb = ctx.enter_context(tc.tile_pool(name="p_wb", bufs=1, space="PSUM"))

    # ---- startup.  Every dma_start costs ~650ns of trigger time on its
    # issuing engine and queues serialize, so the ramp is paced by trigger
    # count per queue and total early bytes.  Spread the weight chunks
    # across the scalar+gpsimd queues (one tile per hc chunk => the first
    # matmul depends only on chunk 0), keep the sync queue for keys, and
    # warm the PE clock (HAM) with zero matmuls on a memset tile while
    # the first transfers land. ----
    uaw = []
    with tc.high_priority():
        scratch = consts.tile([128, 512], BF, name="scratch", bufs=1)
        nc.vector.memset(scratch[:], 0.0)
        dummy = p_wb.tile([128, 512], F32, tag="wb", name="dummy")
        for i in range(8):
            nc.tensor.matmul(dummy[:], scratch[:, 0:128], scratch[:],
                             start=True, stop=True)
        for hc in range(HC):
            w = consts.tile([128, H], BF, name=f"uaw{hc}")
            eng = nc.scalar if hc % 2 == 0 else nc.gpsimd
            eng.dma_start(w[:], d["uawT"][:, hc * H : (hc + 1) * H])
            uaw.append(w)
        biasT = consts.tile([128, HC * BC], F32)
        nc.gpsimd.dma_start(biasT[:], d["biasT"])
        vaT = consts.tile([128, HC], BF)
        nc.gpsimd.dma_start(vaT[:], d["vaT"])
        ones_r = consts.tile([1, 128], BF)
        nc.gpsimd.dma_start(ones_r[:], d["ones"])
        # keys for tiles 0 and 1, in strict sync-queue priority order:
        # tile 0 in pair-chunks (hc-major warmup granularity), tile 1
        # whole.  Later tiles ride the steady-state pipeline; pulling
        # them forward floods HBM and starves this critical path.
        pre_kT = {}
        kT0p = []
        for j in range(HC // 2):
            pk = kpool.tile([128, 2, 512], BF, name=f"kT0p{j}", bufs=1)
            nc.sync.dma_start(pk[:], d["keysTr"][0, :, 2 * j : 2 * j + 2, 0:512])
            kT0p.append(pk)
        pre_kT[0] = [kT0p[hc // 2][:, hc % 2, :] for hc in range(HC)]
        kT1 = kpool.tile([128, HC, 512], BF, name="kT1", bufs=1)
        nc.sync.dma_start(kT1[:], d["keysTr"][0, :, :, 512:1024])
        pre_kT[1] = [kT1[:, hc, :] for hc in range(HC)]

    # Per-batch state, created lazily inside the flat tile loop.
    bstate = {}

    def batch_state(b):
        if b not in bstate:
            mb = small.tile([1, L], F32, tag="mb")
            nc.gpsimd.dma_start(mb[:], d["maskb"][b : b + 1, :])
            s_all = small.tile([1, NSLOT], F32, tag="s_all", name=f"s_all_{b}")
            pp_all = small.tile([128, HC, NSLOT], F32, tag="pp_all",
                                name=f"pp_all_{b}")
            bstate[b] = {"mb": mb, "s_all": s_all, "pp_all": pp_all}
        return bstate[b]

    def emit_scores(rec, kc):
        """One deferred score matmul for tile rec at chunk kc (its tanh is
        a full tile old, so this never stalls the PE)."""
        nc.tensor.matmul(
            rec["ps"][:], vaT[:, kc : kc + 1], rec["tks"][kc][:],
            start=(kc == 0), stop=(kc == HC - 1),
        )

    def emit_softmax(rec):
        """Mask add + exp(+sum) for tile rec; DVE/ACT only.  No max
        subtraction: scores are bounded by ||va||_1 << fp32 exp range."""
        b, l0, lw = rec["b"], rec["l0"], rec["lw"]
        st = bstate[b]
        sm = small.tile([1, lw], F32, tag="sm")
        nc.vector.tensor_add(sm[:], rec["ps"][:],
                             st["mb"][0:1, l0 : l0 + lw])
        e = small.tile([1, lw], BF, tag="e", bufs=3, name=f"e_{b}_{rec['lt']}")
        nc.scalar.activation(e[:], sm[:], AF.Exp, bias=0.0, scale=1.0,
                             accum_out=st["s_all"][0:1, rec["slot"] : rec["slot"] + 1])
        rec["e"] = e

    def emit_wbcast(rec):
        """Partition-broadcast of the exp weights: tiny PE ones-matmul,
        then an ACT copy out of PSUM into a bf16 SBUF tile."""
        lw = rec["lw"]
        wb = p_wb.tile([128, lw], F32, tag="wb")
        nc.tensor.matmul(wb[:], ones_r[:], rec["e"][:], start=True, stop=True)
        wbs = small.tile([128, lw], BF, tag="wbs", bufs=2,
                         name=f"wbs_{rec['b']}_{rec['lt']}")
        nc.scalar.activation(wbs[:], wb[:], AF.Copy)
        rec["wbs"] = wbs

    def emit_wsum(rec):
        """DVE weighted key sum against the resident keysTr tile."""
        b = rec["b"]
        st = bstate[b]
        for hc in range(HC):
            dump = small.tile([128, rec["lw"]], BF, tag="dump")
            nc.vector.scalar_tensor_tensor(
                dump[:],
                rec["kTs"][hc],
                1.0,
                rec["wbs"][:],
                op0=OP.mult,
                op1=OP.mult,
                accum_out=st["pp_all"][:, hc, rec["slot"] : rec["slot"] + 1],
            )
        if rec["last"]:
            ns = rec["slot"] + 1
            acc = small.tile([128, HC], F32, tag="acc")
            nc.vector.tensor_reduce(acc[:], st["pp_all"][:, :, 0:ns],
                                    axis=AX.X, op=OP.add)
            nc.gpsimd.dma_start(d["accout"][b, :, :], acc[:])
            nc.gpsimd.dma_start(d["sout"][b : b + 1, 0:ns], st["s_all"][0:1, 0:ns])

    # tile plan: flat list of (b, l0, lw)
    plan = []
    for b in range(BC):
        l0 = 0
        for i, w in enumerate(WIDTHS[b]):
            plan.append({"b": b, "lt": i, "slot": i, "l0": l0, "lw": w,
                         "last": i == len(WIDTHS[b]) - 1})
            l0 += w

    tiles = []
    for t, rec in enumerate(plan):
        b, l0, lw = rec["b"], rec["l0"], rec["lw"]
        batch_state(b)
        if t in pre_kT:
            kTs = pre_kT[t]
        else:
            kT = kpool.tile([128, HC, lw], BF, tag="kT")
            nc.sync.dma_start(kT[:, :, :], d["keysTr"][b, :, :, l0 : l0 + lw])
            kTs = [kT[:, hc, :] for hc in range(HC)]
        ps = p_sc.tile([1, lw], F32, tag="ps")
        rec.update({"kTs": kTs, "tks": [], "ps": ps})

        def mm(pu, kc, hc):
            nc.tensor.matmul(
                pu[:],
                uaw[hc][:, kc * 128 : (kc + 1) * 128],
                kTs[hc],
                start=(hc == 0),
                stop=(hc == HC - 1),
            )

        def tanh(kc, pu):
            tk = tpool.tile([128, lw], BF, tag="tk")
            nc.scalar.activation(
                tk[:], pu[:], AF.Tanh,
                bias=biasT[:, kc * BC + b : kc * BC + b + 1], scale=1.0,
            )
            rec["tks"].append(tk)

        if t == 0:
            # warm-up: hc-major over kc 0..3 (4 open PSUM accumulations),
            # so compute starts as soon as uawT[hc0]+kT0[hc0] land.
            pus = [p_uk.tile([128, lw], F32, tag="pu", name=f"pu_w{kc}")
                   for kc in range(4)]
            for hc in range(HC):
                for kc in range(4):
                    mm(pus[kc], kc, hc)
            # pass B: kc 4..7 kc-major (weights all resident by now),
            # pipelined with pass A's tanhs.
            for kc in range(4, HC):
                pu = p_uk.tile([128, lw], F32, tag="pu")
                for hc in range(HC):
                    mm(pu, kc, hc)
                tanh(kc - 4, pus[kc - 4])
                pus.append(pu)
            for kc in range(4, HC):
                tanh(kc, pus[kc])
        else:
            for kc in range(HC):
                pu = p_uk.tile([128, lw], F32, tag="pu")
                for hc in range(HC):
                    mm(pu, kc, hc)
                tanh(kc, pu)
                prev = tiles[t - 1]
                if kc < 4:
                    emit_scores(prev, 2 * kc)
                    emit_scores(prev, 2 * kc + 1)
                elif kc == 4:
                    emit_softmax(prev)
                elif kc == 5:
                    emit_wbcast(prev)
                elif kc == HC - 1:
                    emit_wsum(prev)
                if t == len(plan) - 1 and kc >= 1:
                    # final tile: its own scores lag one kc slot so the
                    # flush chain is only tanh(7) -> sc(7) -> exp -> wsum
                    emit_scores(rec, kc - 1)
        tiles.append(rec)

    # flush the final tile
    last = tiles[-1]
    emit_scores(last, HC - 1)
    emit_softmax(last)
    emit_wbcast(last)
    emit_wsum(last)


def build():
    nc = bacc.Bacc("TRN2", target_bir_lowering=False, debug=False,
                   num_devices=NCORES)
    d = {
        "keysTr": nc.dram_tensor("keysTr", [BC, 128, HC, L], BF, kind="ExternalInput").ap(),
        "uawT": nc.dram_tensor("uawT", [128, HC * H], BF, kind="ExternalInput").ap(),
        "vaT": nc.dram_tensor("vaT", [128, HC], BF, kind="ExternalInput").ap(),
        "biasT": nc.dram_tensor("biasT", [128, HC * BC], F32, kind="ExternalInput").ap(),
        "ones": nc.dram_tensor("ones", [1, 128], BF, kind="ExternalInput").ap(),
        "maskb": nc.dram_tensor("maskb", [BC, L], F32, kind="ExternalInput").ap(),
        "accout": nc.dram_tensor("accout", [BC, 128, HC], F32, kind="ExternalOutput").ap(),
        "sout": nc.dram_tensor("sout", [BC, NSLOT], F32, kind="ExternalOutput").ap(),
    }
    with tile.TileContext(nc) as tc, ExitStack() as ctx:
        _body(nc, tc, ctx, d)
    nc.compile()
    return nc


def _maybe_install_profile_hook():
    """BASS_TRACE=1 profiling under axon needs antenv.axon_hooks, which this
    image lacks; shim it with an in-memory module wired to libaxon_pjrt."""
    import sys, types
    if "antenv.axon_hooks" in sys.modules:
        return
    mod = types.ModuleType("antenv.axon_hooks")
    holder = [None]
    mod.set_axon_ntff_profile_hook = lambda h: holder.__setitem__(0, h)
    mod.get_axon_ntff_profile_hook = lambda: holder[0]
    sys.modules["antenv.axon_hooks"] = mod
    try:
        from trn_agent_boot.trn_boot import _ntff_profile_via_ctypes
        mod.set_axon_ntff_profile_hook(
            _ntff_profile_via_ctypes("/opt/axon/libaxon_pjrt.so"))
    except Exception:
        pass


def make_in_maps(query, keys, mask, wa_w, wa_b, ua_w, ua_b, va_w, va_b):
    bf16 = ml_dtypes.bfloat16
    query = np.asarray(query, dtype=np.float32)
    keys = np.asarray(keys, dtype=np.float32)
    mask = np.asarray(mask)
    wa_w = np.asarray(wa_w, dtype=np.float32)
    wa_b = np.asarray(wa_b, dtype=np.float32)
    ua_b = np.asarray(ua_b, dtype=np.float32)
    ua_w = np.asarray(ua_w, dtype=np.float32)
    va_w = np.asarray(va_w, dtype=np.float32)

    # lhsT chunk layout: arr[p, hc*H + k] = W[k, hc*128 + p]
    uawT = np.ascontiguousarray(
        ua_w.T.reshape(HC, 128, H).transpose(1, 0, 2).reshape(128, HC * H)
    ).astype(bf16)
    vaT = np.ascontiguousarray(va_w[0].reshape(HC, 128).T).astype(bf16)
    maskb = np.where(mask, np.float32(-1e30), np.float32(0.0)).astype(np.float32)
    # keysTr[b, p, hc, l] = keys[b, l, hc*128+p]
    keysTr = np.ascontiguousarray(
        keys.transpose(0, 2, 1).reshape(B, HC, 128, L).transpose(0, 2, 1, 3)
    ).astype(bf16)
    # wq + wa_b + ua_b on host (0.05% of the FLOPs)
    wq = query[:, 0, :] @ wa_w.T + wa_b + ua_b  # [B, H]

    in_maps = []
    for c in range(NCORES):
        bs = slice(c * BC, (c + 1) * BC)
        biasT = np.ascontiguousarray(
            wq[bs].T.reshape(HC, 128, BC).transpose(1, 0, 2).reshape(128, HC * BC))
        in_maps.append({
            "keysTr": keysTr[bs],
            "uawT": uawT,
            "vaT": vaT,
            "biasT": biasT,
            "ones": np.ones((1, 128), dtype=bf16),
            "maskb": np.ascontiguousarray(maskb[bs]),
        })
    return in_maps


def kernel(query, keys, mask, wa_w, wa_b, ua_w, ua_b, va_w, va_b):
    global _nc, LAST_RESULT
    if os.environ.get("BASS_TRACE"):
        _maybe_install_profile_hook()
    if _nc is None:
        _nc = build()
    in_maps = make_in_maps(query, keys, mask, wa_w, wa_b, ua_w, ua_b, va_w, va_b)
    res = run_bass_kernel_spmd(_nc, in_maps, list(range(NCORES)))
    LAST_RESULT = res
    outs = []
    for c in range(NCORES):
        acc = res.results[c]["accout"]          # [BC, 128, HC] = ctx^T unnormalized
        sout = res.results[c]["sout"]  # [BC, NSLOT]; batch i uses len(WIDTHS[i]) slots
        ssum = np.array([sout[i, : len(WIDTHS[i])].sum() for i in range(BC)])
        # ctx[b, hc*128+p] = acc[b, p, hc] / ssum[b]
        ctx = acc.transpose(0, 2, 1).reshape(BC, H) / ssum[:, None]
        outs.append(ctx)
    out = np.concatenate(outs, axis=0)
    return np.ascontiguousarray(out[:, None, :].astype(np.float32))


# revision 19
# speedup vs baseline: 1.1361x; 1.0318x over previous
"""Bahdanau additive attention on 8 Trainium2 NeuronCores (Bass/Tile).

Reference computation (per batch b):
    wq   = query @ wa_w.T + wa_b                      # [1, H]
    uk   = keys  @ ua_w.T + ua_b                      # [L, H]
    s    = tanh(wq + uk) @ va_w.T + va_b              # [L]
    s    = where(mask, -inf, s)
    w    = softmax(s)                                 # [L]
    ctx  = w @ keys                                   # [1, H]

Sharding: data-parallel over batch B=32 -> 4 batches per core; small
weights replicated.  The heavy matmul runs on the PE in bf16 (same
78.6 TF/s PE rate as fp32r, but half the DMA/SBUF footprint and 4x
faster weight loads via FWL; accuracy ~1e-3 rel, well inside 2e-2).

Device-side structure (per core, BC=4 batches):
  - keys arrive pre-transposed (host) as keysTr [128, HC, L] per batch;
    the big matmul computes uk^T [k, l] so the per-batch
    wq[k]+wa_b[k]+ua_b[k] (tiny, host-precomputed) is a per-partition
    ACT bias fused into the tanh.
  - scores = va . tanh(.) is a PE matmul with va as a [128,1] stationary.
  - softmax needs NO max subtraction: |scores| <= ||va||_1 ~ 26 << 88,
    so exp never overflows fp32.  exp + per-tile sum fuse into one ACT
    op (accum_out); softmax shift invariance drops va_b.
  - the weighted key sum ctx^T = sum_l e_l * keysT[:, l] runs on the DVE
    (scalar_tensor_tensor multiply with accum_out) against the SAME
    keysTr tiles pass 1 just consumed -> keys are read from HBM once.
    exp weights are partition-broadcast via a tiny PE ones-matmul.
  - tile 0 consumes weights hc-major (4 open PSUM accumulations) so the
    first matmul needs only uawT[hc0]+kT0[hc0] (~384KB) instead of the
    whole weight set; the PE starts ~1.5us in and HAM warms early.
  - per-batch outputs are the unnormalized ctx^T [128, HC] and the
    per-tile exp sums; the host divides by their total and transposes
    during the gather/unshard step (a 32 KB epilogue).
"""

import os
import numpy as np
from contextlib import ExitStack

import ml_dtypes

import concourse.bass as bass  # noqa: F401
import concourse.bacc as bacc
import concourse.tile as tile
from concourse import mybir
from concourse.bass_utils import run_bass_kernel_spmd

B, L, H = 32, 2048, 1024
NCORES = 8
BC = B // NCORES          # batches per core
HC = H // 128             # 128-chunks of the hidden dim
NSLOT = 8                 # slot-dim padding (last batch uses 5 slots)

# l-tile widths per batch; last batch tapers so (a) the second-to-last
# tile's DVE weighted-sum hides under the last tile's matmul stream and
# (b) the serial flush chain after the final matmul is short.
WIDTHS = [[512, 512, 512, 512]] * (BC - 1) + [[512, 512, 384, 384, 256]]

F32 = mybir.dt.float32
BF = mybir.dt.bfloat16
AF = mybir.ActivationFunctionType
AX = mybir.AxisListType
OP = mybir.AluOpType

_nc = None
LAST_RESULT = None


def _body(nc, tc, ctx, d):
    consts = ctx.enter_context(tc.tile_pool(name="consts", bufs=1))
    kpool = ctx.enter_context(tc.tile_pool(name="kT", bufs=8))
    tpool = ctx.enter_context(tc.tile_pool(name="tk", bufs=12))
    small = ctx.enter_context(tc.tile_pool(name="small", bufs=2))
    p_uk = ctx.enter_context(tc.tile_pool(name="p_uk", bufs=5, space="PSUM"))
    p_sc = ctx.enter_context(tc.tile_pool(name="p_sc", bufs=2, space="PSUM"))
    p_wb = ctx.enter_context(tc.tile_pool(name="p_wb", bufs=1, space="PSUM"))

    # ---- startup.  Every dma_start costs ~650ns of trigger time on its
    # issuing engine and queues serialize, so the ramp is paced by trigger
    # count per queue and total early bytes.  Spread the weight chunks
    # across the scalar+gpsimd queues (one tile per hc chunk => the first
    # matmul depends only on chunk 0), keep the sync queue for keys, and
    # warm the PE clock (HAM) with zero matmuls on a memset tile while
    # the first transfers land. ----
    uaw = []
    with tc.high_priority():
        scratch = consts.tile([128, 512], BF, name="scratch", bufs=1)
        nc.vector.memset(scratch[:], 0.0)
        dummy = p_wb.tile([128, 512], F32, tag="wb", name="dummy")
        for i in range(8):
            nc.tensor.matmul(dummy[:], scratch[:, 0:128], scratch[:],
                             start=True, stop=True)
        for hc in range(HC):
            w = consts.tile([128, H], BF, name=f"uaw{hc}")
            eng = nc.scalar if hc % 2 == 0 else nc.gpsimd
            eng.dma_start(w[:], d["uawT"][:, hc * H : (hc + 1) * H])
            uaw.append(w)
        biasT = consts.tile([128, HC * BC], F32)
        nc.gpsimd.dma_start(biasT[:], d["biasT"])
        vaT = consts.tile([128, HC], BF)
        nc.gpsimd.dma_start(vaT[:], d["vaT"])
        ones_r = consts.tile([1, 128], BF)
        nc.gpsimd.dma_start(ones_r[:], d["ones"])
        # keys for tiles 0 and 1, in strict sync-queue priority order:
        # tile 0 in pair-chunks (hc-major warmup granularity), tile 1
        # whole.  Later tiles ride the steady-state pipeline; pulling
        # them forward floods HBM and starves this critical path.
        pre_kT = {}
        kT0p = []
        for j in range(HC // 2):
            pk = kpool.tile([128, 2, 512], BF, name=f"kT0p{j}", bufs=1)
            nc.sync.dma_start(pk[:], d["keysTr"][0, :, 2 * j : 2 * j + 2, 0:512])
            kT0p.append(pk)
        pre_kT[0] = [kT0p[hc // 2][:, hc % 2, :] for hc in range(HC)]
        kT1 = kpool.tile([128, HC, 512], BF, name="kT1", bufs=1)
        nc.sync.dma_start(kT1[:], d["keysTr"][0, :, :, 512:1024])
        pre_kT[1] = [kT1[:, hc, :] for hc in range(HC)]

    # Per-batch state, created lazily inside the flat tile loop.
    bstate = {}

    def batch_state(b):
        if b not in bstate:
            mb = small.tile([1, L], F32, tag="mb")
            nc.gpsimd.dma_start(mb[:], d["maskb"][b : b + 1, :])
            s_all = small.tile([1, NSLOT], F32, tag="s_all", name=f"s_all_{b}")
            pp_all = small.tile([128, HC, NSLOT], F32, tag="pp_all",
                                name=f"pp_all_{b}")
            bstate[b] = {"mb": mb, "s_all": s_all, "pp_all": pp_all}
        return bstate[b]

    def emit_scores(rec, j):
        """Deferred score matmuls for kc pair (2j, 2j+1): two concurrent
        column-tiled matmuls (col groups 0 and 32 of the PE array each
        stream their own tanh chunk), halving the PE slots scores cost."""
        nc.tensor.matmul(
            rec["ps"][0:1, :], vaT[:, 2 * j : 2 * j + 1], rec["tks"][2 * j][:],
            start=(j == 0), stop=(j == HC // 2 - 1), tile_position=(0, 0),
        )
        nc.tensor.matmul(
            rec["ps"][32:33, :], vaT[:, 2 * j + 1 : 2 * j + 2],
            rec["tks"][2 * j + 1][:],
            start=(j == 0), stop=(j == HC // 2 - 1), tile_position=(0, 32),
        )

    def emit_softmax(rec):
        """Combine the two col-group partial scores + mask, then exp with
        fused tile-sum.  No max subtraction: scores are bounded by
        ||va||_1 << fp32 exp range."""
        b, l0, lw = rec["b"], rec["l0"], rec["lw"]
        st = bstate[b]
        s2 = small.tile([1, lw], F32, tag="s2")
        nc.vector.tensor_add(s2[:], rec["ps"][0:1, :],
                             st["mb"][0:1, l0 : l0 + lw])
        sm = small.tile([1, lw], F32, tag="sm")
        nc.vector.tensor_add(sm[:], s2[:], rec["ps"][32:33, :])
        e = small.tile([1, lw], BF, tag="e", bufs=3, name=f"e_{b}_{rec['lt']}")
        nc.scalar.activation(e[:], sm[:], AF.Exp, bias=0.0, scale=1.0,
                             accum_out=st["s_all"][0:1, rec["slot"] : rec["slot"] + 1])
        rec["e"] = e

    def emit_wbcast(rec):
        """Partition-broadcast of the exp weights: tiny PE ones-matmul,
        then an ACT copy out of PSUM into a bf16 SBUF tile."""
        lw = rec["lw"]
        wb = p_wb.tile([128, lw], F32, tag="wb")
        nc.tensor.matmul(wb[:], ones_r[:], rec["e"][:], start=True, stop=True)
        wbs = small.tile([128, lw], BF, tag="wbs", bufs=2,
                         name=f"wbs_{rec['b']}_{rec['lt']}")
        nc.scalar.activation(wbs[:], wb[:], AF.Copy)
        rec["wbs"] = wbs

    def emit_wsum(rec):
        """DVE weighted key sum against the resident keysTr tile."""
        b = rec["b"]
        st = bstate[b]
        for hc in range(HC):
            dump = small.tile([128, rec["lw"]], BF, tag="dump")
            nc.vector.scalar_tensor_tensor(
                dump[:],
                rec["kTs"][hc],
                1.0,
                rec["wbs"][:],
                op0=OP.mult,
                op1=OP.mult,
                accum_out=st["pp_all"][:, hc, rec["slot"] : rec["slot"] + 1],
            )
        if rec["last"]:
            ns = rec["slot"] + 1
            acc = small.tile([128, HC], F32, tag="acc")
            nc.vector.tensor_reduce(acc[:], st["pp_all"][:, :, 0:ns],
                                    axis=AX.X, op=OP.add)
            nc.gpsimd.dma_start(d["accout"][b, :, :], acc[:])
            nc.gpsimd.dma_start(d["sout"][b : b + 1, 0:ns], st["s_all"][0:1, 0:ns])

    # tile plan: flat list of (b, l0, lw)
    plan = []
    for b in range(BC):
        l0 = 0
        for i, w in enumerate(WIDTHS[b]):
            plan.append({"b": b, "lt": i, "slot": i, "l0": l0, "lw": w,
                         "last": i == len(WIDTHS[b]) - 1})
            l0 += w

    tiles = []
    for t, rec in enumerate(plan):
        b, l0, lw = rec["b"], rec["l0"], rec["lw"]
        batch_state(b)
        if t in pre_kT:
            kTs = pre_kT[t]
        else:
            kT = kpool.tile([128, HC, lw], BF, tag="kT")
            nc.sync.dma_start(kT[:, :, :], d["keysTr"][b, :, :, l0 : l0 + lw])
            kTs = [kT[:, hc, :] for hc in range(HC)]
        ps = p_sc.tile([33, lw], F32, tag="ps")
        rec.update({"kTs": kTs, "tks": [], "ps": ps})

        def mm(pu, kc, hc):
            nc.tensor.matmul(
                pu[:],
                uaw[hc][:, kc * 128 : (kc + 1) * 128],
                kTs[hc],
                start=(hc == 0),
                stop=(hc == HC - 1),
            )

        def tanh(kc, pu):
            tk = tpool.tile([128, lw], BF, tag="tk")
            nc.scalar.activation(
                tk[:], pu[:], AF.Tanh,
                bias=biasT[:, kc * BC + b : kc * BC + b + 1], scale=1.0,
            )
            rec["tks"].append(tk)

        if t == 0:
            # warm-up: hc-major over kc 0..3 (4 open PSUM accumulations),
            # so compute starts as soon as uawT[hc0]+kT0[hc0] land.
            pus = [p_uk.tile([128, lw], F32, tag="pu", name=f"pu_w{kc}")
                   for kc in range(4)]
            for hc in range(HC):
                for kc in range(4):
                    mm(pus[kc], kc, hc)
            # pass B: kc 4..7 kc-major (weights all resident by now),
            # pipelined with pass A's tanhs.
            for kc in range(4, HC):
                pu = p_uk.tile([128, lw], F32, tag="pu")
                for hc in range(HC):
                    mm(pu, kc, hc)
                tanh(kc - 4, pus[kc - 4])
                pus.append(pu)
            for kc in range(4, HC):
                tanh(kc, pus[kc])
        else:
            for kc in range(HC):
                pu = p_uk.tile([128, lw], F32, tag="pu")
                for hc in range(HC):
                    mm(pu, kc, hc)
                tanh(kc, pu)
                prev = tiles[t - 1]
                if kc < 4:
                    emit_scores(prev, kc)
                elif kc == 4:
                    emit_softmax(prev)
                    emit_wbcast(prev)
                elif kc == 5:
                    emit_wsum(prev)
                if t == len(plan) - 1 and kc >= 2 and kc % 2 == 0:
                    # final tile: its own score pairs lag so the flush
                    # chain is only tanh(7) -> sc(6,7) -> exp -> wsum
                    emit_scores(rec, kc // 2 - 1)
        tiles.append(rec)

    # flush the final tile
    last = tiles[-1]
    emit_scores(last, HC // 2 - 1)
    emit_softmax(last)
    emit_wbcast(last)
    emit_wsum(last)


def build():
    nc = bacc.Bacc("TRN2", target_bir_lowering=False, debug=False,
                   num_devices=NCORES)
    d = {
        "keysTr": nc.dram_tensor("keysTr", [BC, 128, HC, L], BF, kind="ExternalInput").ap(),
        "uawT": nc.dram_tensor("uawT", [128, HC * H], BF, kind="ExternalInput").ap(),
        "vaT": nc.dram_tensor("vaT", [128, HC], BF, kind="ExternalInput").ap(),
        "biasT": nc.dram_tensor("biasT", [128, HC * BC], F32, kind="ExternalInput").ap(),
        "ones": nc.dram_tensor("ones", [1, 128], BF, kind="ExternalInput").ap(),
        "maskb": nc.dram_tensor("maskb", [BC, L], F32, kind="ExternalInput").ap(),
        "accout": nc.dram_tensor("accout", [BC, 128, HC], F32, kind="ExternalOutput").ap(),
        "sout": nc.dram_tensor("sout", [BC, NSLOT], F32, kind="ExternalOutput").ap(),
    }
    with tile.TileContext(nc) as tc, ExitStack() as ctx:
        _body(nc, tc, ctx, d)
    nc.compile()
    return nc


def _maybe_install_profile_hook():
    """BASS_TRACE=1 profiling under axon needs antenv.axon_hooks, which this
    image lacks; shim it with an in-memory module wired to libaxon_pjrt."""
    import sys, types
    if "antenv.axon_hooks" in sys.modules:
        return
    mod = types.ModuleType("antenv.axon_hooks")
    holder = [None]
    mod.set_axon_ntff_profile_hook = lambda h: holder.__setitem__(0, h)
    mod.get_axon_ntff_profile_hook = lambda: holder[0]
    sys.modules["antenv.axon_hooks"] = mod
    try:
        from trn_agent_boot.trn_boot import _ntff_profile_via_ctypes
        mod.set_axon_ntff_profile_hook(
            _ntff_profile_via_ctypes("/opt/axon/libaxon_pjrt.so"))
    except Exception:
        pass


def make_in_maps(query, keys, mask, wa_w, wa_b, ua_w, ua_b, va_w, va_b):
    bf16 = ml_dtypes.bfloat16
    query = np.asarray(query, dtype=np.float32)
    keys = np.asarray(keys, dtype=np.float32)
    mask = np.asarray(mask)
    wa_w = np.asarray(wa_w, dtype=np.float32)
    wa_b = np.asarray(wa_b, dtype=np.float32)
    ua_b = np.asarray(ua_b, dtype=np.float32)
    ua_w = np.asarray(ua_w, dtype=np.float32)
    va_w = np.asarray(va_w, dtype=np.float32)

    # lhsT chunk layout: arr[p, hc*H + k] = W[k, hc*128 + p]
    uawT = np.ascontiguousarray(
        ua_w.T.reshape(HC, 128, H).transpose(1, 0, 2).reshape(128, HC * H)
    ).astype(bf16)
    vaT = np.ascontiguousarray(va_w[0].reshape(HC, 128).T).astype(bf16)
    maskb = np.where(mask, np.float32(-1e30), np.float32(0.0)).astype(np.float32)
    # keysTr[b, p, hc, l] = keys[b, l, hc*128+p]
    keysTr = np.ascontiguousarray(
        keys.transpose(0, 2, 1).reshape(B, HC, 128, L).transpose(0, 2, 1, 3)
    ).astype(bf16)
    # wq + wa_b + ua_b on host (0.05% of the FLOPs)
    wq = query[:, 0, :] @ wa_w.T + wa_b + ua_b  # [B, H]

    in_maps = []
    for c in range(NCORES):
        bs = slice(c * BC, (c + 1) * BC)
        biasT = np.ascontiguousarray(
            wq[bs].T.reshape(HC, 128, BC).transpose(1, 0, 2).reshape(128, HC * BC))
        in_maps.append({
            "keysTr": keysTr[bs],
            "uawT": uawT,
            "vaT": vaT,
            "biasT": biasT,
            "ones": np.ones((1, 128), dtype=bf16),
            "maskb": np.ascontiguousarray(maskb[bs]),
        })
    return in_maps


def kernel(query, keys, mask, wa_w, wa_b, ua_w, ua_b, va_w, va_b):
    global _nc, LAST_RESULT
    if os.environ.get("BASS_TRACE"):
        _maybe_install_profile_hook()
    if _nc is None:
        _nc = build()
    in_maps = make_in_maps(query, keys, mask, wa_w, wa_b, ua_w, ua_b, va_w, va_b)
    res = run_bass_kernel_spmd(_nc, in_maps, list(range(NCORES)))
    LAST_RESULT = res
    outs = []
    for c in range(NCORES):
        acc = res.results[c]["accout"]          # [BC, 128, HC] = ctx^T unnormalized
        sout = res.results[c]["sout"]  # [BC, NSLOT]; batch i uses len(WIDTHS[i]) slots
        ssum = np.array([sout[i, : len(WIDTHS[i])].sum() for i in range(BC)])
        # ctx[b, hc*128+p] = acc[b, p, hc] / ssum[b]
        ctx = acc.transpose(0, 2, 1).reshape(BC, H) / ssum[:, None]
        outs.append(ctx)
    out = np.concatenate(outs, axis=0)
    return np.ascontiguousarray(out[:, None, :].astype(np.float32))


# revision 22
# speedup vs baseline: 1.6787x; 1.4777x over previous
"""Bahdanau additive attention on 8 Trainium2 NeuronCores (Bass/Tile).

Reference computation (per batch b):
    wq   = query @ wa_w.T + wa_b                      # [1, H]
    uk   = keys  @ ua_w.T + ua_b                      # [L, H]
    s    = tanh(wq + uk) @ va_w.T + va_b              # [L]
    s    = where(mask, -inf, s)
    w    = softmax(s)                                 # [L]
    ctx  = w @ keys                                   # [1, H]

Sharding: data-parallel over batch B=32 -> 4 batches per core; small
weights replicated.  The heavy matmul runs on the PE in bf16 (same
78.6 TF/s PE rate as fp32r, but half the DMA/SBUF footprint and 4x
faster weight loads via FWL; accuracy ~1e-3 rel, well inside 2e-2).

Device-side structure (per core, BC=4 batches):
  - keys arrive pre-transposed (host) as keysTr [128, HC, L] per batch;
    the big matmul computes uk^T [k, l] so the per-batch
    wq[k]+wa_b[k]+ua_b[k] (tiny, host-precomputed) is a per-partition
    ACT bias fused into the tanh.
  - scores = va . tanh(.) is a PE matmul with va as a [128,1] stationary.
  - softmax needs NO max subtraction: |scores| <= ||va||_1 ~ 26 << 88,
    so exp never overflows fp32.  exp + per-tile sum fuse into one ACT
    op (accum_out); softmax shift invariance drops va_b.
  - the weighted key sum ctx^T = sum_l e_l * keysT[:, l] runs on the DVE
    (scalar_tensor_tensor multiply with accum_out) against the SAME
    keysTr tiles pass 1 just consumed -> keys are read from HBM once.
    exp weights are partition-broadcast via a tiny PE ones-matmul.
  - tile 0 consumes weights hc-major (4 open PSUM accumulations) so the
    first matmul needs only uawT[hc0]+kT0[hc0] (~384KB) instead of the
    whole weight set; the PE starts ~1.5us in and HAM warms early.
  - per-batch outputs are the unnormalized ctx^T [128, HC] and the
    per-tile exp sums; the host divides by their total and transposes
    during the gather/unshard step (a 32 KB epilogue).
"""

import os
import numpy as np
from contextlib import ExitStack

import ml_dtypes

import concourse.bass as bass  # noqa: F401
import concourse.bacc as bacc
import concourse.tile as tile
from concourse import mybir
from concourse.bass_utils import run_bass_kernel_spmd

B, L, H = 32, 2048, 1024
NCORES = 8
BC = B // NCORES          # batches per core
HC = H // 128             # 128-chunks of the hidden dim
NSLOT = 8                 # slot-dim padding (last batch uses 5 slots)

# l-tile widths per batch; last batch tapers so (a) the second-to-last
# tile's DVE weighted-sum hides under the last tile's matmul stream and
# (b) the serial flush chain after the final matmul is short.
WIDTHS = [[512, 512, 512, 512]] * (BC - 1) + [[512, 512, 384, 384, 256]]

F32 = mybir.dt.float32
BF = mybir.dt.bfloat16
F8 = mybir.dt.float8e4
DR = mybir.MatmulPerfMode.DoubleRow
HCP = HC // 2             # DoubleRow processes hc chunks in pairs
AF = mybir.ActivationFunctionType
AX = mybir.AxisListType
OP = mybir.AluOpType

_nc = None
LAST_RESULT = None


def _body(nc, tc, ctx, d):
    consts = ctx.enter_context(tc.tile_pool(name="consts", bufs=1))
    kpool = ctx.enter_context(tc.tile_pool(name="kT", bufs=8))
    tpool = ctx.enter_context(tc.tile_pool(name="tk", bufs=12))
    small = ctx.enter_context(tc.tile_pool(name="small", bufs=2))
    p_uk = ctx.enter_context(tc.tile_pool(name="p_uk", bufs=5, space="PSUM"))
    p_sc = ctx.enter_context(tc.tile_pool(name="p_sc", bufs=2, space="PSUM"))
    p_wb = ctx.enter_context(tc.tile_pool(name="p_wb", bufs=1, space="PSUM"))

    # ---- startup.  Every dma_start costs ~650ns of trigger time on its
    # issuing engine and queues serialize, so the ramp is paced by trigger
    # count per queue and total early bytes.  Spread the weight chunks
    # across the scalar+gpsimd queues (one tile per hc chunk => the first
    # matmul depends only on chunk 0), keep the sync queue for keys, and
    # warm the PE clock (HAM) with zero matmuls on a memset tile while
    # the first transfers land. ----
    uaw8 = []
    with tc.high_priority():
        scratch = consts.tile([128, 512], BF, name="scratch", bufs=1)
        nc.vector.memset(scratch[:], 0.0)
        dummy = p_wb.tile([128, 512], F32, tag="wb", name="dummy")
        for i in range(8):
            nc.tensor.matmul(dummy[:], scratch[:, 0:128], scratch[:],
                             start=True, stop=True)
        for hcp in range(HCP):
            w = consts.tile([128, 2, H], F8, name=f"uaw8_{hcp}")
            eng = nc.scalar if hcp % 2 == 0 else nc.gpsimd
            eng.dma_start(w[:], d["uaw8"][:, hcp, :, :])
            uaw8.append(w)
        biasT = consts.tile([128, HC * BC], F32)
        nc.gpsimd.dma_start(biasT[:], d["biasT"])
        vaT = consts.tile([128, HC], BF)
        nc.gpsimd.dma_start(vaT[:], d["vaT"])
        ones_r = consts.tile([1, 128], BF)
        nc.gpsimd.dma_start(ones_r[:], d["ones"])
        # keys for tiles 0 and 1, in strict sync-queue priority order:
        # tile 0 in pair-chunks (hc-major warmup granularity), tile 1
        # whole.  Later tiles ride the steady-state pipeline; pulling
        # them forward floods HBM and starves this critical path.
        pre_k8 = {}
        k80p = []
        for j in range(HCP):
            pk = kpool.tile([128, 2, 512], F8, name=f"k80p{j}", bufs=1)
            nc.sync.dma_start(pk[:], d["keys8"][0, :, j, :, 0:512])
            k80p.append(pk)
        pre_k8[0] = [k80p[j][:, :, :] for j in range(HCP)]
        k81 = kpool.tile([128, HCP, 2, 512], F8, name="k81", bufs=1)
        nc.sync.dma_start(k81[:], d["keys8"][0, :, :, :, 512:1024])
        pre_k8[1] = [k81[:, j, :, :] for j in range(HCP)]
        pre_kT = {}
        for t, (l0n, lwn) in enumerate([(0, 512), (512, 512)]):
            kTn = kpool.tile([128, HC, 512], BF, name=f"kTpre{t}", bufs=1)
            nc.sync.dma_start(kTn[:], d["keysTr"][0, :, :, l0n : l0n + lwn])
            pre_kT[t] = [kTn[:, hc, :] for hc in range(HC)]

    # Per-batch state, created lazily inside the flat tile loop.
    bstate = {}

    def batch_state(b):
        if b not in bstate:
            mb = small.tile([1, L], F32, tag="mb")
            nc.gpsimd.dma_start(mb[:], d["maskb"][b : b + 1, :])
            s_all = small.tile([1, NSLOT], F32, tag="s_all", name=f"s_all_{b}")
            pp_all = small.tile([128, HC, NSLOT], F32, tag="pp_all",
                                name=f"pp_all_{b}")
            bstate[b] = {"mb": mb, "s_all": s_all, "pp_all": pp_all}
        return bstate[b]

    def emit_scores(rec, j):
        """Deferred score matmuls for kc pair (2j, 2j+1): two concurrent
        column-tiled matmuls (col groups 0 and 32 of the PE array each
        stream their own tanh chunk), halving the PE slots scores cost."""
        nc.tensor.matmul(
            rec["ps"][0:1, :], vaT[:, 2 * j : 2 * j + 1], rec["tks"][2 * j][:],
            start=(j == 0), stop=(j == HC // 2 - 1), tile_position=(0, 0),
        )
        nc.tensor.matmul(
            rec["ps"][32:33, :], vaT[:, 2 * j + 1 : 2 * j + 2],
            rec["tks"][2 * j + 1][:],
            start=(j == 0), stop=(j == HC // 2 - 1), tile_position=(0, 32),
        )

    def emit_softmax(rec):
        """Combine the two col-group partial scores + mask, then exp with
        fused tile-sum.  No max subtraction: scores are bounded by
        ||va||_1 << fp32 exp range."""
        b, l0, lw = rec["b"], rec["l0"], rec["lw"]
        st = bstate[b]
        s2 = small.tile([1, lw], F32, tag="s2")
        nc.vector.tensor_add(s2[:], rec["ps"][0:1, :],
                             st["mb"][0:1, l0 : l0 + lw])
        sm = small.tile([1, lw], F32, tag="sm")
        nc.vector.tensor_add(sm[:], s2[:], rec["ps"][32:33, :])
        e = small.tile([1, lw], BF, tag="e", bufs=3, name=f"e_{b}_{rec['lt']}")
        nc.scalar.activation(e[:], sm[:], AF.Exp, bias=0.0, scale=1.0,
                             accum_out=st["s_all"][0:1, rec["slot"] : rec["slot"] + 1])
        rec["e"] = e

    def emit_wbcast(rec):
        """Partition-broadcast of the exp weights: tiny PE ones-matmul,
        then an ACT copy out of PSUM into a bf16 SBUF tile."""
        lw = rec["lw"]
        wb = p_wb.tile([128, lw], F32, tag="wb")
        nc.tensor.matmul(wb[:], ones_r[:], rec["e"][:], start=True, stop=True)
        wbs = small.tile([128, lw], BF, tag="wbs", bufs=2,
                         name=f"wbs_{rec['b']}_{rec['lt']}")
        nc.scalar.activation(wbs[:], wb[:], AF.Copy)
        rec["wbs"] = wbs

    def emit_wsum(rec):
        """DVE weighted key sum against the resident keysTr tile."""
        b = rec["b"]
        st = bstate[b]
        for hc in range(HC):
            dump = small.tile([128, rec["lw"]], BF, tag="dump")
            nc.vector.scalar_tensor_tensor(
                dump[:],
                rec["kTs"][hc],
                1.0,
                rec["wbs"][:],
                op0=OP.mult,
                op1=OP.mult,
                accum_out=st["pp_all"][:, hc, rec["slot"] : rec["slot"] + 1],
            )
        if rec["last"]:
            ns = rec["slot"] + 1
            acc = small.tile([128, HC], F32, tag="acc")
            nc.vector.tensor_reduce(acc[:], st["pp_all"][:, :, 0:ns],
                                    axis=AX.X, op=OP.add)
            nc.gpsimd.dma_start(d["accout"][b, :, :], acc[:])
            nc.gpsimd.dma_start(d["sout"][b : b + 1, 0:ns], st["s_all"][0:1, 0:ns])

    # tile plan: flat list of (b, l0, lw)
    plan = []
    for b in range(BC):
        l0 = 0
        for i, w in enumerate(WIDTHS[b]):
            plan.append({"b": b, "lt": i, "slot": i, "l0": l0, "lw": w,
                         "last": i == len(WIDTHS[b]) - 1})
            l0 += w

    tiles = []
    for t, rec in enumerate(plan):
        b, l0, lw = rec["b"], rec["l0"], rec["lw"]
        batch_state(b)
        if t in pre_k8:
            k8s = pre_k8[t]
            kTs = pre_kT[t]
        else:
            k8 = kpool.tile([128, HCP, 2, lw], F8, tag="k8")
            nc.sync.dma_start(k8[:, :, :, :], d["keys8"][b, :, :, :, l0 : l0 + lw])
            kT = kpool.tile([128, HC, lw], BF, tag="kT")
            nc.sync.dma_start(kT[:, :, :], d["keysTr"][b, :, :, l0 : l0 + lw])
            k8s = [k8[:, j, :, :] for j in range(HCP)]
            kTs = [kT[:, hc, :] for hc in range(HC)]
        ps = p_sc.tile([33, lw], F32, tag="ps")
        rec.update({"kTs": kTs, "tks": [], "ps": ps})

        def mm(pu, kc, hcp):
            nc.tensor.matmul(
                pu[:],
                uaw8[hcp][:, :, kc * 128 : (kc + 1) * 128],
                k8s[hcp],
                start=(hcp == 0),
                stop=(hcp == HCP - 1),
                perf_mode=DR,
            )

        def tanh(kc, pu):
            tk = tpool.tile([128, lw], BF, tag="tk")
            nc.scalar.activation(
                tk[:], pu[:], AF.Tanh,
                bias=biasT[:, kc * BC + b : kc * BC + b + 1], scale=1.0,
            )
            rec["tks"].append(tk)

        if t == 0:
            # warm-up: hcp-major over kc 0..3 (4 open PSUM accumulations),
            # so compute starts as soon as uaw8[0]+k80[0] land.
            pus = [p_uk.tile([128, lw], F32, tag="pu", name=f"pu_w{kc}")
                   for kc in range(4)]
            for hcp in range(HCP):
                for kc in range(4):
                    mm(pus[kc], kc, hcp)
            # pass B: kc 4..7 kc-major (weights all resident by now),
            # pipelined with pass A's tanhs.
            for kc in range(4, HC):
                pu = p_uk.tile([128, lw], F32, tag="pu")
                for hcp in range(HCP):
                    mm(pu, kc, hcp)
                tanh(kc - 4, pus[kc - 4])
                pus.append(pu)
            for kc in range(4, HC):
                tanh(kc, pus[kc])
        else:
            for kc in range(HC):
                pu = p_uk.tile([128, lw], F32, tag="pu")
                for hcp in range(HCP):
                    mm(pu, kc, hcp)
                tanh(kc, pu)
                prev = tiles[t - 1]
                if kc < 4:
                    emit_scores(prev, kc)
                elif kc == 4:
                    emit_softmax(prev)
                    emit_wbcast(prev)
                elif kc == 5:
                    emit_wsum(prev)
                if t == len(plan) - 1 and kc >= 2 and kc % 2 == 0:
                    # final tile: its own score pairs lag so the flush
                    # chain is only tanh(7) -> sc(6,7) -> exp -> wsum
                    emit_scores(rec, kc // 2 - 1)
        tiles.append(rec)

    # flush the final tile
    last = tiles[-1]
    emit_scores(last, HC // 2 - 1)
    emit_softmax(last)
    emit_wbcast(last)
    emit_wsum(last)


def build():
    nc = bacc.Bacc("TRN2", target_bir_lowering=False, debug=False,
                   num_devices=NCORES)
    d = {
        "keysTr": nc.dram_tensor("keysTr", [BC, 128, HC, L], BF, kind="ExternalInput").ap(),
        "keys8": nc.dram_tensor("keys8", [BC, 128, HCP, 2, L], F8, kind="ExternalInput").ap(),
        "uaw8": nc.dram_tensor("uaw8", [128, HCP, 2, H], F8, kind="ExternalInput").ap(),
        "vaT": nc.dram_tensor("vaT", [128, HC], BF, kind="ExternalInput").ap(),
        "biasT": nc.dram_tensor("biasT", [128, HC * BC], F32, kind="ExternalInput").ap(),
        "ones": nc.dram_tensor("ones", [1, 128], BF, kind="ExternalInput").ap(),
        "maskb": nc.dram_tensor("maskb", [BC, L], F32, kind="ExternalInput").ap(),
        "accout": nc.dram_tensor("accout", [BC, 128, HC], F32, kind="ExternalOutput").ap(),
        "sout": nc.dram_tensor("sout", [BC, NSLOT], F32, kind="ExternalOutput").ap(),
    }
    with tile.TileContext(nc) as tc, ExitStack() as ctx:
        _body(nc, tc, ctx, d)
    nc.compile()
    return nc


def _maybe_install_profile_hook():
    """BASS_TRACE=1 profiling under axon needs antenv.axon_hooks, which this
    image lacks; shim it with an in-memory module wired to libaxon_pjrt."""
    import sys, types
    if "antenv.axon_hooks" in sys.modules:
        return
    mod = types.ModuleType("antenv.axon_hooks")
    holder = [None]
    mod.set_axon_ntff_profile_hook = lambda h: holder.__setitem__(0, h)
    mod.get_axon_ntff_profile_hook = lambda: holder[0]
    sys.modules["antenv.axon_hooks"] = mod
    try:
        from trn_agent_boot.trn_boot import _ntff_profile_via_ctypes
        mod.set_axon_ntff_profile_hook(
            _ntff_profile_via_ctypes("/opt/axon/libaxon_pjrt.so"))
    except Exception:
        pass


def make_in_maps(query, keys, mask, wa_w, wa_b, ua_w, ua_b, va_w, va_b):
    bf16 = ml_dtypes.bfloat16
    query = np.asarray(query, dtype=np.float32)
    keys = np.asarray(keys, dtype=np.float32)
    mask = np.asarray(mask)
    wa_w = np.asarray(wa_w, dtype=np.float32)
    wa_b = np.asarray(wa_b, dtype=np.float32)
    ua_b = np.asarray(ua_b, dtype=np.float32)
    ua_w = np.asarray(ua_w, dtype=np.float32)
    va_w = np.asarray(va_w, dtype=np.float32)

    fp8 = ml_dtypes.float8_e4m3
    # DoubleRow lhsT layout: arr[p, hcp, j, k] = W[k, (2*hcp+j)*128 + p]
    uaw8 = np.ascontiguousarray(
        ua_w.T.reshape(HCP, 2, 128, H).transpose(2, 0, 1, 3)
    ).astype(fp8)
    vaT = np.ascontiguousarray(va_w[0].reshape(HC, 128).T).astype(bf16)
    maskb = np.where(mask, np.float32(-1e30), np.float32(0.0)).astype(np.float32)
    # keysTr[b, p, hc, l] = keys[b, l, hc*128+p]  (bf16, weighted sum)
    keysT = keys.transpose(0, 2, 1)  # [B, H, L]
    keysTr = np.ascontiguousarray(
        keysT.reshape(B, HC, 128, L).transpose(0, 2, 1, 3)
    ).astype(bf16)
    # keys8[b, p, hcp, j, l] = keys[b, l, (2*hcp+j)*128+p]  (fp8, matmul)
    keys8 = np.ascontiguousarray(
        keysT.reshape(B, HCP, 2, 128, L).transpose(0, 3, 1, 2, 4)
    ).astype(fp8)
    # wq + wa_b + ua_b on host (0.05% of the FLOPs)
    wq = query[:, 0, :] @ wa_w.T + wa_b + ua_b  # [B, H]

    in_maps = []
    for c in range(NCORES):
        bs = slice(c * BC, (c + 1) * BC)
        biasT = np.ascontiguousarray(
            wq[bs].T.reshape(HC, 128, BC).transpose(1, 0, 2).reshape(128, HC * BC))
        in_maps.append({
            "keysTr": keysTr[bs],
            "keys8": keys8[bs],
            "uaw8": uaw8,
            "vaT": vaT,
            "biasT": biasT,
            "ones": np.ones((1, 128), dtype=bf16),
            "maskb": np.ascontiguousarray(maskb[bs]),
        })
    return in_maps


def kernel(query, keys, mask, wa_w, wa_b, ua_w, ua_b, va_w, va_b):
    global _nc, LAST_RESULT
    if os.environ.get("BASS_TRACE"):
        _maybe_install_profile_hook()
    if _nc is None:
        _nc = build()
    in_maps = make_in_maps(query, keys, mask, wa_w, wa_b, ua_w, ua_b, va_w, va_b)
    res = run_bass_kernel_spmd(_nc, in_maps, list(range(NCORES)))
    LAST_RESULT = res
    outs = []
    for c in range(NCORES):
        acc = res.results[c]["accout"]          # [BC, 128, HC] = ctx^T unnormalized
        sout = res.results[c]["sout"]  # [BC, NSLOT]; batch i uses len(WIDTHS[i]) slots
        ssum = np.array([sout[i, : len(WIDTHS[i])].sum() for i in range(BC)])
        # ctx[b, hc*128+p] = acc[b, p, hc] / ssum[b]
        ctx = acc.transpose(0, 2, 1).reshape(BC, H) / ssum[:, None]
        outs.append(ctx)
    out = np.concatenate(outs, axis=0)
    return np.ascontiguousarray(out[:, None, :].astype(np.float32))
